# revision 13
# baseline (speedup 1.0000x reference)
"""EIF neuron kernel for Trainium2 (Bass/Tile), 8-core data-parallel.

Reference semantics (TAU=1.0, V_TH=1.0, DELTA_T=0.2, V_RESET=0.0):
    e      = 0.2 * exp((m - 1) / 0.2)
    m'     = m + (x_t - m + e) / 1.0   == x_t + e
    spike  = (m' >= 1)
    m      = where(spike, 0, m')

The whole step runs on the VECTOR ENGINE ONLY as two chained custom-DVE
instructions (no ACT, no cross-engine semaphores in the serial chain):

    e == 0.2*e^{5(m-1)} == f(m)^32,  f(m) = 2^((K*m + AC)/32),
    K = 5/ln2, AC = log2(0.2) - K.  On the occupied domain m in (-6, 1),
    f is approximated by a monic cubic in v = m+1 (rel err 3.1e-6,
    e-term rel err ~1e-4), evaluated and raised to the 32nd power by
    repeated squaring across the two 8-stage instructions:

    INSTR1 (EIF_POLY_ANT):  v = m+1;  q = ((v+c2)v+c1)v+c0;  out = q^4
    INSTR2 (EIF_STEP_ANT):  e = (((q^4)^2 * a3^8)^2)^2   [= (a3*q)^32]
                            m' = e + x;   out = (m' < 1) * m'

    The cubic stays positive and small for v <= 0, so e <= ~1e-5 there
    -- it clamps itself; no explicit clamp stage or C3 spill is needed
    (3 coefficients ride the 3 scalar slots; INSTR1 is single-stream).

out doubles as the stored state for the next step AND the DMA'd result;
the host recovers spikes as bits(out) == +0.0 (the reset writes +0.0;
a no-spike m' is +-nonzero except for exact-zero sums, which are
measure-zero in fp32).  Measured fidelity vs the fp32 jax reference:
~25 flipped spikes of 67M (budget at rel_err 2e-2 is ~4300).

Sharding: batch dim B=32 -> 4 batches per core; per core the (b,n) set
is 128 partitions x 128 free columns per timestep; T=512 serial steps.
The per-step serial chain is broken into chains=2 independent 64-col
column groups interleaved [1X,1Y,2X,2Y] so every RAW dependency is two
instructions apart and the DVE overlaps SBUF access latency with the
neighbor chain's execution (measured 256 -> 168us end-to-end).  DMA
in/out in 64-step chunks, double-buffered (DMA floor is ~45us/rep --
nowhere near binding).
"""

import numpy as np
from contextlib import ExitStack

import concourse.bass as bass
import concourse.bacc as bacc
import concourse.tile as tile
from concourse import mybir
from concourse.bass_utils import run_bass_kernel_spmd

F32 = mybir.dt.float32

B, T, N = 32, 512, 4096
NCORES = 8
BPC = B // NCORES            # 4 batches per core
P = 128                      # SBUF partitions
FD = (BPC * N) // P          # 128 free columns per timestep
TC = 64                      # timesteps per DMA chunk

# Cubic fit of f(v) = 2^((K*(v-1) + AC)/32) over v in [0, 2] (v = m+1),
# monic p/a3 = ((v + C2F)*v + C1F)*v + C0F; fitted with iteratively-
# reweighted relative LS on Chebyshev nodes (fit_poly3.py).  e-term =
# (a3*p)^32; rel err ~1e-4 -> ~25 flipped spikes of 67M vs the reference.
A3 = np.float32(0.0005165081820450723)
C2F = np.float32(16.262502670288086)
C1F = np.float32(210.53695678710938)
C0F = np.float32(1346.9814453125)
_A3_2 = np.float32(A3 * A3)
_A3_4 = np.float32(_A3_2 * _A3_2)
LAM = np.float32(_A3_4 * _A3_4)                               # a3^8

_registered = None
_built = None


def _f32(x):
    return np.asarray(x, np.float32)


def _ref_poly(in0, in1, s0, s1, imm2):
    """out = (((v+s0)v+s1)v+imm2)^4-chain start: q then q^2 then q^4; v=in0+1."""
    f32 = np.float32
    v = _f32(in0.astype(f32) + f32(1.0))
    h = _f32(v + f32(s0))
    h = _f32(_f32(h * v) + f32(s1))
    q = _f32(_f32(h * v) + f32(imm2))
    q2 = _f32(q * q)
    return _f32(q2 * q2)


def _ref_step(in0, in1, s0, s1, imm2):
    """out = (m'<1)*m', m' = (((in0^2)*s1)^2)^2 + in1 (elementwise)."""
    f32 = np.float32
    q4 = in0.astype(f32)
    x = np.asarray(in1, f32).reshape(q4.shape[0], -1)
    t = _f32(q4 * q4)
    tl = _f32(t * f32(s1))
    t2 = _f32(tl * tl)
    e = _f32(t2 * t2)
    mp = _f32(e + x.reshape(q4.shape))
    return _f32((mp < f32(1.0)).astype(f32) * mp)


def _register_ops():
    """Register the two EIF custom-DVE ops in concourse.dve_ops (the
    documented extension point -- appended, never reordered)."""
    global _registered
    if _registered is not None:
        return _registered
    from concourse import dve_ops
    from concourse.dve_spec import (
        Spec, Src0, Src1, C0, C1, C2, One, lower, _has_src1,
    )
    from concourse.dve_uop import DveOpSpec

    v = Src0 + One
    q = ((v + C0) * v + C1) * v + C2
    q2 = q * q
    spec_poly = Spec(body=q2 * q2, reference=_ref_poly)

    t = Src0 * Src0
    tl = t * C1
    t2 = tl * tl
    e = t2 * t2
    mp = e + Src1
    spec_step = Spec(body=(mp < One) * mp, reference=_ref_step)

    ops = []
    for name, spec in (("EIF_POLY_ANT", spec_poly), ("EIF_STEP_ANT", spec_step)):
        existing = [o for o in dve_ops.OPS if o.name == name]
        if existing:
            ops.append(existing[0])
            continue
        row = dve_ops._CUSTOM_DVE_ROW_BASE + len(dve_ops.OPS)
        shas = {}
        for ver in ("v3", "v4"):
            try:
                uops = lower(spec, ver=ver)
            except Exception:
                continue
            shas[ver] = DveOpSpec(
                name=name, opcode=row, uops=uops, rd1_en=_has_src1(spec)
            ).sha(ver)
        op = dve_ops.DveOp(name, spec, subdim=False, uops_sha=shas)
        dve_ops.OPS.append(op)
        dve_ops.CUSTOM_DVE_SPECS[name] = spec
        dve_ops._SUB_OPCODE_FOR_NAME[name] = row
        assert row < 0x20
        ops.append(op)
    _registered = tuple(ops)
    return _registered


def _build(reps=1, tc=TC, xbufs=2, sbufs=2, chains=2, xmem="sbuf", mtc=8,
           mring=4):
    """chains=1: one 128-col chain, every DVE instr depends on the previous
    (pipeline drains between them).  chains=2: two independent 64-col chains
    interleaved [1X,1Y,2X,2Y] so every RAW is 2 instructions apart and the
    engine can overlap access latency with the neighbor chain.
    xmem="psum": stream x into a PSUM ring (mring mini-chunks of mtc steps)
    so instr2 reads its two operands from different memories."""
    op_poly, op_step = _register_ops()
    nc = bacc.Bacc("TRN2", debug=False, num_devices=NCORES)
    x_d = nc.declare_dram_parameter("x", [P, T * FD], F32, isOutput=False)
    s_d = nc.declare_dram_parameter("spk", [P, T * FD], F32, isOutput=True)

    gf = FD // chains
    with ExitStack() as ctx:
        tcx = ctx.enter_context(tile.TileContext(nc))
        spool = ctx.enter_context(tcx.tile_pool(name="sout", bufs=sbufs))
        state = ctx.enter_context(tcx.tile_pool(name="state", bufs=1))
        xpool = None
        if xmem == "sbuf":
            xpool = ctx.enter_context(tcx.tile_pool(name="xin", bufs=xbufs))
        else:
            xr = [nc.alloc_psum_tensor([P, mtc * FD], F32, name=f"px{i}")
                  for i in range(mring)]
            xr3 = [t[:].rearrange("p (t f) -> p t f", f=FD) for t in xr]

        mz = state.tile([P, FD], F32, name="mz", tag="mz")
        nc.vector.memset(mz[:], 0.0)
        q = [state.tile([P, gf], F32, name=f"q{g}", tag=f"q{g}")
             for g in range(chains)]

        def _mdma(mi):
            nc.sync.dma_start(
                out=xr[mi % mring][:],
                in_=x_d[:, mi * mtc * FD:(mi + 1) * mtc * FD],
            )

        prev = [mz[:, g * gf:(g + 1) * gf] for g in range(chains)]
        for _rep in range(reps):
            if xmem != "sbuf":
                for j in range(mring - 1):
                    _mdma(j)
            for ci in range(T // tc):
                if xmem == "sbuf":
                    xt = xpool.tile([P, tc * FD], F32, name="xt", tag="x")
                    nc.sync.dma_start(
                        out=xt[:], in_=x_d[:, ci * tc * FD:(ci + 1) * tc * FD]
                    )
                    xt3 = xt.rearrange("p (t f) -> p t f", f=FD)
                sp = spool.tile([P, tc * FD], F32, name="sp", tag="s")
                sp3 = sp.rearrange("p (t f) -> p t f", f=FD)

                for k in range(tc):
                    if xmem != "sbuf" and k % mtc == 0:
                        tgt = (ci * tc + k) // mtc + mring - 1
                        if tgt < T // mtc:
                            _mdma(tgt)
                    for g in range(chains):
                        # INSTR1: q4 = cubic(m_prev + 1)^4
                        nc.vector._custom_dve(
                            op_poly, out=q[g][:], in0=prev[g],
                            s0=float(C2F), s1=float(C1F), imm2=float(C0F),
                        )
                    for g in range(chains):
                        if xmem == "sbuf":
                            xin = xt3[:, k:k + 1, g * gf:(g + 1) * gf]
                        else:
                            mi = (ci * tc + k) // mtc
                            xin = xr3[mi % mring][:, k % mtc:k % mtc + 1,
                                                  g * gf:(g + 1) * gf]
                        # INSTR2: out = (m'<1)*m', m' = (a3*cubic)^32 + x
                        nc.vector._custom_dve(
                            op_step, out=sp3[:, k, g * gf:(g + 1) * gf],
                            in0=q[g][:], in1=xin, s1=float(LAM),
                        )
                        prev[g] = sp3[:, k, g * gf:(g + 1) * gf]
                nc.sync.dma_start(
                    out=s_d[:, ci * tc * FD:(ci + 1) * tc * FD], in_=sp[:]
                )
    nc.compile()
    return nc


def _shard(x):
    """x[B,T,N] -> per-core [P, T*FD] partition-major arrays."""
    maps = []
    for c in range(NCORES):
        xc = x[c * BPC:(c + 1) * BPC]                      # [4, T, 4096]
        xc = np.ascontiguousarray(
            xc.reshape(BPC, T, N // FD, FD).transpose(0, 2, 1, 3)
        ).reshape(P, T * FD)
        maps.append({"x": xc})
    return maps


def _unshard(results):
    out = np.empty((B, T, N), np.float32)
    for c in range(NCORES):
        r = np.ascontiguousarray(np.asarray(results[c]["spk"]))
        r = r.reshape(BPC, N // FD, T, FD).transpose(0, 2, 1, 3).reshape(BPC, T, N)
        # spike fired iff the reset wrote +0.0 (bit-exact test)
        out[c * BPC:(c + 1) * BPC] = (r.view(np.uint32) == 0).astype(np.float32)
    return out


def kernel(x):
    global _built
    x = np.asarray(x, dtype=np.float32)
    assert x.shape == (B, T, N), x.shape
    if _built is None:
        _built = _build()
    res = run_bass_kernel_spmd(_built, _shard(x), list(range(NCORES)))
    return _unshard(res.results)
